# revision 1
# baseline (speedup 1.0000x reference)
"""MCANet forward on 8 Trainium2 NeuronCores (Bass/Tile), data-parallel over batch.

Per core: 4 samples (LD=512, LP=4096, H=128). Key idea: the row/col max
reductions over the [512, 4096] affinity matrix (the baseline's Vector-engine
bottleneck) are replaced by a log-sum-exp max approximation computed on the
otherwise-idle Scalar (ACT) engine:

    max_i x_i  ~=  ln(sum_i exp(k*x_i)) / k          (k = 1024)

|aff| <~ 0.03 so k*aff stays within the exp range, exp-sums stay far inside
the ACT Ln table's valid input range (~2^64), and the LSE error
log(n_eff)/k <~ 8e-3 perturbs the (nearly uniform) softmax weights far below
the 2e-2 tolerance.

Per sample (pipelined across samples; the ACT engine is the bottleneck and
runs at ~85% occupancy):
  PE   : aff tiles [m=128p, l=512f] = pfT_chunk^T @ dfT  (orientation B only)
  ACT  : E = exp(k*aff) PSUM->SBUF bf16 (one op per [128,1024] PSUM block);
         DVE takes every 4th block via the Schraudolph bit-trick exp
         (int16 bits = k*aff*128/ln2 + 127*128, reinterpreted as bf16)
  DVE  : colsum[m] = sum_l E[m, l] via tensor_scalar+accum_out (4x bf16 mode)
  PE   : rowsum[l] = sum_m E[m, l] via E-chunk-stationary x ones matmuls
         into one dedicated PSUM bank as a SINGLE long accumulation group
         (one start marks the bank's zero-region once; emitted with a
         2-block lag so they never stall the PE behind an exp)
  tail : w = 1 + ln(sum)/k  (~ sum^(1/k) ~ exp(max)); weighted feature sums,
         denominators, reciprocal broadcast (via ones-matmul), and the MLP,
         split into 7 stages run one-per-block under the NEXT sample's
         blocks so no engine stalls on cross-engine dependencies.

Host packs gathered embeddings into [dfT|pfT] / [dfn|pfn] tensors (fewer,
earlier-starting DMAs), shards over cores, concatenates per-core outputs.
"""

import os
import sys

sys.path.insert(0, "/opt/trn_rl_repo")
_HERE = os.path.dirname(os.path.abspath(__file__))
if _HERE not in sys.path:
    sys.path.insert(0, _HERE)

import numpy as np
import ml_dtypes

import concourse.bass as bass
import concourse.tile as tile
from concourse import mybir
from concourse.bass_utils import run_bass_kernel_spmd

F32 = mybir.dt.float32
BF16 = mybir.dt.bfloat16
AF = mybir.ActivationFunctionType
ALU = mybir.AluOpType
NCORES = 8
B, LD, LP, H = 32, 512, 4096, 128
SPC = B // NCORES  # samples per core
NMT = LP // 128    # 32 m-tiles per sample
NLT = LD // 128    # 4 l-subtiles
KSCALE = 1024.0    # LSE sharpness; keeps exp-sums well inside the ACT
                   # engine's Ln table range (~2^64)

# PSUM blocks: [128, 1024] fp32 x 3 bufs (6 banks) + 1 bank rowsum
# accumulator + 1 bank misc = 8 banks total. Three slots let the PE write
# affinity blocks ahead of BOTH exp consumers (ACT + DVE).
BLKW = 1024
BLOCKS = [(j, 2) for j in range(0, 32, 2)]
# sample 0 starts cold: tiny first blocks so the first exp issues early
BLOCKS0 = [(0, 1), (1, 1)] + [(j, 2) for j in range(2, 32, 2)]
ROW_LAG = 2  # blocks of lag before a block's rowsum matmuls are emitted

# Work split across engines. DVE computes exp for some blocks with the
# Schraudolph bit trick: int16 bits = round(k*aff*(128/ln2) + 127*128 - 4)
# reinterpreted as bf16 ~= exp(k*aff) within ~3% (harmless: it perturbs
# ln(sum) by <0.03 -> weights by 3e-5). Pool cannot help: the walrus ISA
# check rejects both PSUM access and TensorScalarPtr on Pool.
SCHR_C0 = KSCALE * 128.0 / float(np.log(2.0))
SCHR_C1 = 127.0 * 128.0 - 4.0
DVE_EXP_BLOCKS = {3, 7, 11, 15}    # ~8 of 32 tiles per sample (16-block lists)
POOL_CS_BLOCKS = set()             # Pool is ISA-locked out of TensorScalarPtr

_MAX_WAITS = int(os.environ.get("KERNEL_MAX_WAITS", "1"))


def _split_excess_waits(nc, max_waits=_MAX_WAITS):
    """This walrus build rejects instructions carrying more than ~2 sync
    waits ("Too many sync wait commands"). Hoist excess waits onto injected
    same-engine NOPs placed immediately before the instruction — engines
    execute their streams in order, so the waits still gate it."""
    import bass_rust

    cnt = 0
    for bb in nc.main_func.blocks:
        old = list(bb.instructions)
        need = any(
            ins.sync_info is not None and len(ins.sync_info.on_wait) > max_waits
            for ins in old
        )
        if not need:
            continue
        new = []
        for ins in old:
            si = ins.sync_info
            waits = list(si.on_wait) if si is not None else []
            if len(waits) > max_waits:
                chunks = [
                    waits[i : i + max_waits] for i in range(0, len(waits), max_waits)
                ]
                for ch in chunks[:-1]:
                    nop = mybir.InstNoOp(name=f"wsplit_{cnt}", ins=[], outs=[])
                    cnt += 1
                    nop.engine = ins.engine
                    nop.sync_info = bass_rust.SyncInfo(on_wait=ch, on_update=[])
                    new.append(nop)
                ins.sync_info = bass_rust.SyncInfo(
                    on_wait=chunks[-1], on_update=si.on_update
                )
            new.append(ins)
        bb.instructions = new
    return cnt


class _SplitDrainTileContext(tile.TileContext):
    def _drain_and_barrier(self, tick_clock, wait_clock):
        super()._drain_and_barrier(tick_clock, wait_clock)
        n = _split_excess_waits(self.nc)
        print(f"[kernel] split {n} excess-wait chunks onto nops")


def _build_nc():
    nc = bass.Bass()
    fT_d = nc.declare_dram_parameter("fT", [SPC, 128, LD + LP], BF16, isOutput=False)
    fn_d = nc.declare_dram_parameter(
        "fn", [SPC, 128, NLT + NMT, 128], BF16, isOutput=False
    )
    w1_d = nc.declare_dram_parameter("w1", [2 * H, 64], F32, isOutput=False)
    b1_d = nc.declare_dram_parameter("b1", [64], F32, isOutput=False)
    w2_d = nc.declare_dram_parameter("w2", [64, 1], F32, isOutput=False)
    b2_d = nc.declare_dram_parameter("b2", [1], F32, isOutput=False)
    out_d = nc.declare_dram_parameter("out", [SPC, 1], F32, isOutput=True)

    with _SplitDrainTileContext(nc) as tc:
        with (
            tc.tile_pool(name="feat", bufs=3) as feat,
            tc.tile_pool(name="epool", bufs=5) as epool,
            tc.tile_pool(name="singles", bufs=1) as singles,
            tc.tile_pool(name="stats", bufs=2) as stats,
            tc.tile_pool(name="blk", bufs=3, space="PSUM") as blk,
            tc.tile_pool(name="prow", bufs=1, space="PSUM") as prow,
            tc.tile_pool(name="misc", bufs=1, space="PSUM") as misc,
        ):
            ones = singles.tile([128, 1], BF16)
            nc.vector.memset(ones, 1.0)
            ones_row = singles.tile([1, 128], F32)
            nc.vector.memset(ones_row, 1.0)
            outs_sb = singles.tile([1, SPC], F32)
            dump = singles.tile([128, 512], BF16)  # tensor_scalar main-out sink
            nc.vector.memset(dump, 0.0)
            dumq = singles.tile([128, 512], BF16)  # separate sink for Pool
            nc.vector.memset(dumq, 0.0)

            tiles = {}

            def load(s):
                # packed [dfT | pfT] in one tile; staged DMAs so the first
                # aff matmuls start after the first small piece lands
                fT = feat.tile([128, LD + LP], BF16, tag="fT")
                nc.sync.dma_start(out=fT[:, :1024], in_=fT_d[s, :, :1024])
                nc.sync.dma_start(out=fT[:, 1024:2560], in_=fT_d[s, :, 1024:2560])
                nc.sync.dma_start(out=fT[:, 2560:], in_=fT_d[s, :, 2560:])
                fn = feat.tile([128, NLT + NMT, 128], BF16, tag="fn")
                nc.sync.dma_start(out=fn, in_=fn_d[s])
                dfT = fT[:, 0:LD]
                pfT = fT[:, LD : LD + LP]
                dfn = fn[:, 0:NLT, :]
                pfn = fn[:, NLT : NLT + NMT, :]
                tiles[s] = (dfT, pfT, pfn, dfn)

            load(0)
            # warm up the Tensor engine during the initial DMA wait so the
            # p-state clock is ramped before the first aff matmuls
            warm = misc.tile([128, 512], F32, tag="pm")
            for _ in range(3):
                nc.tensor.matmul(
                    warm[:1, 0:512], lhsT=ones[:], rhs=dump[:],
                    start=True, stop=True,
                )
            w1_sb = singles.tile([128, 2, 64], F32)
            nc.sync.dma_start(
                out=w1_sb, in_=w1_d.rearrange("(c p) o -> p c o", p=128)
            )
            b1_sb = singles.tile([64, 1], F32)
            nc.sync.dma_start(out=b1_sb, in_=b1_d.rearrange("(p o) -> p o", o=1))
            w2_sb = singles.tile([64, 1], F32)
            nc.sync.dma_start(out=w2_sb, in_=w2_d[:])
            b2_sb = singles.tile([1, 1], F32)
            nc.sync.dma_start(out=b2_sb, in_=b2_d.rearrange("(p o) -> p o", o=1))

            def tail_ln(s, cs, rs):
                """ln of the LSE sums -> attention weights (early part).
                cs[:, 0:NMT] holds colsums, cs[:, NMT:NMT+NLT] the rowsum
                snapshot — one Ln + one weights op covers both. The last
                sample Lns the rowsum psum bank directly instead (rs)."""
                # Exp and Ln share an ACT table set -> no table reload
                lnw = stats.tile([128, NMT + NLT], F32, tag="lnw")
                if rs is not None:
                    nc.scalar.activation(
                        lnw[:, 0:NMT], cs[:, 0:NMT], AF.Ln
                    )
                    nc.scalar.activation(
                        lnw[:, NMT : NMT + NLT], rs[:, 0:NLT], AF.Ln
                    )
                else:
                    nc.scalar.activation(lnw, cs[:], AF.Ln)
                # attention weights w = 1 + ln(sum)/k  (~ sum^(1/k))
                wv = stats.tile([128, NMT + NLT], BF16, tag="wv")
                nc.vector.tensor_scalar(
                    out=wv, in0=lnw, scalar1=1.0 / KSCALE, scalar2=1.0,
                    op0=ALU.mult, op1=ALU.add,
                )
                return wv[:, 0:NMT], wv[:, NMT : NMT + NLT]

            def make_tail(s, cs, pfn, dfn, psP):
                """Per-sample tail as fine-grained stages; each stage's PE
                ops have all cross-engine inputs ready when emitted one or
                more blocks later."""
                st = {}

                def g0():  # ACT: ln; DVE: weights
                    st["wp"], st["wd"] = tail_ln(
                        s, cs, psP if s == SPC - 1 else None
                    )

                def g1():  # PE: denominators + weighted sums (need wp/wd)
                    wp, wd = st["wp"], st["wd"]
                    pm = misc.tile([128, 512], F32, tag="pm")
                    st["pm"] = pm
                    nc.tensor.matmul(
                        pm[:1, 64:96], lhsT=ones[:], rhs=wp[:],
                        start=True, stop=True,
                    )
                    nc.tensor.matmul(
                        pm[:1, 96:100], lhsT=ones[:], rhs=wd[:],
                        start=True, stop=True,
                    )
                    for j in range(NMT):
                        nc.tensor.matmul(
                            pm[:, 1:2],
                            lhsT=pfn[:, j, :],
                            rhs=wp[:, j : j + 1],
                            start=(j == 0),
                            stop=(j == NMT - 1),
                        )
                    for t in range(NLT):
                        nc.tensor.matmul(
                            pm[:, 0:1],
                            lhsT=dfn[:, t, :],
                            rhs=wd[:, t : t + 1],
                            start=(t == 0),
                            stop=(t == NLT - 1),
                        )

                def g2():  # DVE only: dsum, reciprocal, cv copy
                    pm = st["pm"]
                    dsum = stats.tile([1, 2], F32, tag="dsum")
                    nc.vector.reduce_sum(
                        dsum[:1, 1:2], pm[:1, 64:96], axis=mybir.AxisListType.X
                    )
                    nc.vector.reduce_sum(
                        dsum[:1, 0:1], pm[:1, 96:100], axis=mybir.AxisListType.X
                    )
                    rec = stats.tile([1, 2], F32, tag="rec")
                    nc.vector.reciprocal(rec, dsum[:])
                    cv = stats.tile([128, 2], F32, tag="cv")
                    nc.vector.tensor_scalar(
                        out=cv, in0=pm[:, 0:2], scalar1=1.0, scalar2=None,
                        op0=ALU.mult,
                    )
                    st["rec"], st["cv"] = rec, cv

                def g3():  # PE: W1 on unnormalized vectors + rec broadcast
                    pm, rec, cv = st["pm"], st["rec"], st["cv"]
                    nc.tensor.matmul(
                        pm[:64, 128:129], lhsT=w1_sb[:, 0, :], rhs=cv[:, 0:1],
                        start=True, stop=True,
                    )
                    nc.tensor.matmul(
                        pm[:64, 132:133], lhsT=w1_sb[:, 1, :], rhs=cv[:, 1:2],
                        start=True, stop=True,
                    )
                    nc.tensor.matmul(
                        pm[:, 200:202], lhsT=ones_row[:], rhs=rec[:],
                        start=True, stop=True,
                    )

                def g4():  # DVE: h = relu(hd*rSd + hp*rSp + b1)
                    pm = st["pm"]
                    tv = stats.tile([64, 1], F32, tag="tv")
                    nc.vector.tensor_scalar_mul(
                        tv, pm[:64, 128:129], pm[:64, 200:201]
                    )
                    hv = stats.tile([64, 1], F32, tag="hv")
                    nc.vector.scalar_tensor_tensor(
                        out=hv, in0=pm[:64, 132:133], scalar=pm[:64, 201:202],
                        in1=tv[:], op0=ALU.mult, op1=ALU.add,
                    )
                    hb = stats.tile([64, 1], F32, tag="hb")
                    nc.vector.tensor_scalar(
                        out=hb, in0=hv, scalar1=b1_sb[:, 0:1],
                        scalar2=0.0, op0=ALU.add, op1=ALU.max,
                    )
                    st["hb"] = hb

                def g5():  # PE: W2
                    nc.tensor.matmul(
                        st["pm"][:1, 136:137], lhsT=w2_sb[:], rhs=st["hb"][:],
                        start=True, stop=True,
                    )

                def g6():  # DVE: + b2 -> output slot
                    nc.vector.tensor_scalar(
                        out=outs_sb[:, s : s + 1], in0=st["pm"][:1, 136:137],
                        scalar1=b2_sb[:, 0:1], scalar2=None, op0=ALU.add,
                    )

                return [g0, g1, g2, g3, g4, g5, g6]

            # Deferred colsum emission (see block loop).
            csq = []

            def pop_colsums():
                eb_, j0_, nj_, cs_ = csq.pop(0)
                for jj in range(nj_):
                    j = j0_ + jj
                    nc.vector.tensor_scalar(
                        out=dump[:],
                        in0=eb_[:, jj * 512 : (jj + 1) * 512],
                        scalar1=1.0,
                        scalar2=None,
                        op0=ALU.mult,
                        op1=ALU.add,
                        accum_out=cs_[:, j : j + 1],
                    )

            # Deferred rowsum emission: each entry is one block's E tile.
            # All of a sample's rowsum chunk matmuls accumulate into ONE
            # psum bank as a SINGLE long accumulation group (one start on
            # the very first matmul marks the whole bank's zero-region
            # pending, so each column's first write lands on pending bytes
            # and later writes accumulate — interleaved columns are safe).
            rowq = []

            def pop_rows():
                psP, eb, nj, first, last = rowq.pop(0)
                for t in range(NLT):
                    for jj in range(nj):
                        nc.tensor.matmul(
                            psP[:, t : t + 1],
                            lhsT=eb[:, jj * 512 + t * 128 : jj * 512 + (t + 1) * 128],
                            rhs=ones[:],
                            start=bool(first and t == 0 and jj == 0),
                            stop=bool(last and t == NLT - 1 and jj == nj - 1),
                        )
                if last:
                    # snapshot into the colsum tile's trailing columns so
                    # one Ln covers colsums + rowsums; the LAST sample has
                    # no bank-reuse pressure, so it skips the snapshot and
                    # Lns the psum accumulator directly (shorter end chain)
                    cs_, s_ = last
                    if s_ != SPC - 1:
                        nc.vector.tensor_scalar(
                            out=cs_[:, NMT : NMT + NLT], in0=psP[:, 0:NLT],
                            scalar1=1.0, scalar2=None, op0=ALU.mult,
                        )

            stages = []  # pending tail stages of the previous sample
            for s in range(SPC):
                dfT, pfT, pfn, dfn = tiles.pop(s)

                # colsums [128, 0:NMT] + rowsum snapshot [128, NMT:NMT+NLT]
                cs = stats.tile([128, NMT + NLT], F32, tag="cs")
                # rowsum accumulator bank
                psP = prow.tile([128, 512], F32, tag="psP")

                blocks = BLOCKS0 if s == 0 else BLOCKS
                nb = len(blocks)
                for bi, (j0, nj) in enumerate(blocks):
                    w = nj * 512
                    # emit pending rowsum matmuls BEFORE this block's affs:
                    # they are ready (their exp ran blocks ago), so the PE
                    # chews them while waiting for the PSUM slot to free
                    if len(rowq) > ROW_LAG:
                        pop_rows()
                    psB = blk.tile([128, BLKW], F32, tag="psB")
                    for jj in range(nj):
                        j = j0 + jj
                        nc.tensor.matmul(
                            psB[:, jj * 512 : (jj + 1) * 512],
                            lhsT=pfT[:, j * 128 : (j + 1) * 128],
                            rhs=dfT,
                            start=True,
                            stop=True,
                        )
                    eb = epool.tile([128, BLKW], BF16, tag="eb")
                    dve_set = (
                        {3, 7, 11} if s == SPC - 1 else DVE_EXP_BLOCKS
                    )
                    if (bi - (1 if s == 0 else 0)) in dve_set or (
                        s == 0 and bi == 1
                    ):
                        nc.vector.tensor_scalar(
                            out=eb.bitcast(mybir.dt.int16)[:, :w],
                            in0=psB[:, :w], scalar1=SCHR_C0, scalar2=SCHR_C1,
                            op0=ALU.mult, op1=ALU.add,
                        )
                    else:
                        nc.scalar.activation(
                            eb[:, :w], psB[:, :w], AF.Exp, scale=KSCALE
                        )
                    # colsums are deferred by 2 blocks so a DVE-exp op
                    # never sits behind a colsum backlog in the DVE queue
                    # (that would hold the PSUM block slot and stall PE/ACT)
                    csq.append((eb, j0, nj, cs))
                    while len(csq) > 1:
                        pop_colsums()
                    rowq.append(
                        (psP, eb, nj, bi == 0,
                         (cs, s) if bi == nb - 1 else None)
                    )
                    # at the very last block, drain all but the final entry
                    # so only rows(b_last) remain after the loop
                    if s == SPC - 1 and bi == nb - 1:
                        while len(rowq) > 1:
                            pop_rows()
                    # software-pipeline: prefetch next sample's inputs, run
                    # the previous sample's tail under this one's blocks
                    if bi == 0 and s + 1 < SPC:
                        load(s + 1)
                    if bi >= 2 and stages:
                        stages.pop(0)()

                while csq:
                    pop_colsums()
                stages = make_tail(s, cs, pfn, dfn, psP)

            # drain the pipeline
            while rowq:
                pop_rows()
            for g in stages:
                g()
            nc.sync.dma_start(
                out=out_d.rearrange("s o -> o s"), in_=outs_sb[:]
            )
    return nc


_NC_CACHE = None


def kernel(drug_ids, prot_ids, drug_emb, prot_emb, W1, b1, W2, b2):
    global _NC_CACHE
    drug_ids = np.asarray(drug_ids)
    prot_ids = np.asarray(prot_ids)
    drug_emb = np.asarray(drug_emb, dtype=np.float32)
    prot_emb = np.asarray(prot_emb, dtype=np.float32)
    W1 = np.asarray(W1, dtype=np.float32)
    b1 = np.asarray(b1, dtype=np.float32)
    W2 = np.asarray(W2, dtype=np.float32)
    b2 = np.asarray(b2, dtype=np.float32)

    # host-side gather of the small tables into matmul-friendly layouts
    d_feat = drug_emb[drug_ids]  # [B, LD, H]
    p_feat = prot_emb[prot_ids]  # [B, LP, H]
    dfT = d_feat.transpose(0, 2, 1)
    pfT = p_feat.transpose(0, 2, 1)
    fT = np.ascontiguousarray(
        np.concatenate([dfT, pfT], axis=2)
    ).astype(ml_dtypes.bfloat16)  # [B, 128, LD+LP]
    dfn = d_feat.reshape(B, NLT, 128, H).transpose(0, 2, 1, 3)
    pfn = p_feat.reshape(B, NMT, 128, H).transpose(0, 2, 1, 3)
    fn = np.ascontiguousarray(
        np.concatenate([dfn, pfn], axis=2)
    ).astype(ml_dtypes.bfloat16)  # [B, 128, NLT+NMT, H]

    if _NC_CACHE is None:
        _NC_CACHE = _build_nc()
    nc = _NC_CACHE

    in_maps = []
    for c in range(NCORES):
        sl = slice(c * SPC, (c + 1) * SPC)
        in_maps.append(
            {"fT": fT[sl], "fn": fn[sl],
             "w1": W1, "b1": b1, "w2": W2, "b2": b2}
        )

    trace = bool(os.environ.get("KERNEL_TRACE"))
    res = run_bass_kernel_spmd(nc, in_maps, list(range(NCORES)), trace=trace)
    kernel.last_result = res
    out = np.concatenate([res.results[c]["out"] for c in range(NCORES)], axis=0)
    return out.astype(np.float32)


kernel.last_result = None



# revision 2
# speedup vs baseline: 1.0517x; 1.0517x over previous
"""MCANet forward on 8 NeuronCores — vocab-factored exact algorithm, v3.

prot vocab is only 26, so aff[l,m] = G[pid_m, l] with G = prot_emb @ d_feat^T
([26, 512] per sample). Row/col maxes, softmaxes and pooled vectors follow
from G plus per-sample vocab counts c_v (host bincount):

  rowmax[l] = max_{v present} G[v, l]          (Gt orientation, DVE reduce)
  colmax[m] = M[pid_m],  M[v] = max_l G[v, l]
  p_vec = sum_v c_v e^{M_v} emb_v / sum_v c_v e^{M_v}
  d_vec = sum_l e^{rowmax_l} f_l / sum_l e^{rowmax_l}

v3:
 - dfT shipped as fp8e4 (x8 scaled; G is x64 scaled, exp scales folded)
 - sample pair A's M via ACT exp-accumulate LSE (k=1024), pair B via DVE
   reduce_max -> engines balanced
 - transposed MLP tail: relu scale-invariance relu(z/l) = relu(z)/l with
   l = Dd*Dp turns per-sample scalars into per-partition columns; the whole
   post-pool chain is 2 PE hops + DVE-only legs.
"""

import os
import sys

sys.path.insert(0, "/opt/trn_rl_repo")
_HERE = os.path.dirname(os.path.abspath(__file__))
if _HERE not in sys.path:
    sys.path.insert(0, _HERE)

import numpy as np
import ml_dtypes

import concourse.bass as bass
import concourse.tile as tile
from concourse import mybir
from concourse.bass_utils import run_bass_kernel_spmd

F32 = mybir.dt.float32
BF16 = mybir.dt.bfloat16
FP8 = mybir.dt.float8e4
AF = mybir.ActivationFunctionType
ALU = mybir.AluOpType

NCORES = 8
B, LD, LP, H, PV = 32, 512, 4096, 128, 26
SPC = B // NCORES   # 4 samples per core
NLT = LD // 128     # 4 l-tiles
DS = 8.0            # host scale on dfT and pT (fp8 denormal dodge)
SC = DS * DS        # G is SC * G_true
KLSE = 1024.0       # LSE sharpness in true-G units

# ---- blob column layout (bf16 columns) ----
# D0a section: what the G/Gt matmuls need
C_PT = 0            # [128, 13] = [128, 26] fp8 prot_emb^T * 8
C_LNC = 14          # [58, 4] ln(counts) fp32: pairA 14:16, pairB 16:18
C_ONES = 18         # [128, 1] bf16 ones column
C_ONE4B = 20        # [1@p0, 4] bf16 ones row
C_ZERO = 24         # [128, 2] zero f32 column (activation bias)
C_SMA_END = 32
# D0b section: pools + tail constants
C_PEMB = 1344       # [58, 128] prot_emb bf16 (partitions 0:26 and 32:58)
C_W1 = 1472         # [128, 130] W1 * |w2| (65 d-cols then 65 p-cols, col 64/129 pad)
C_B1R = 1602        # [1@p0, 65] bf16: b1 * |w2| with col 64 = |b2|
C_YROW = 1668       # [4, 65] bf16: sign(w2) row, col 64 = sign(b2)
C_ONESRF = 1734     # [1@p0, 256] = [1, 128] f32 ones row
C_MASK = 1990       # [1@p0, 208] fp8 mask rows (-300 if absent), 52 per sample
C_SMB_END = 2198


def C_DFT(s):
    return C_SMA_END + 256 * s              # fp8: 256 bf16-cols = 512 vals


def C_DFN(s):
    return C_SMB_END + 512 * s


D0A_END = C_SMA_END + 256 * SPC   # 1344: smallA + all dfT
D0B_END = C_SMB_END               # 1990: + smallB
NB = C_SMB_END + 512 * SPC        # 4038
_MAX_WAITS = int(os.environ.get("KERNEL_MAX_WAITS", "1"))


def _split_excess_waits(nc, max_waits=_MAX_WAITS):
    """Walrus rejects instructions with more than ~2 sync waits. Hoist excess
    waits onto injected same-engine NOPs immediately before the instruction."""
    import bass_rust

    cnt = 0
    for bb in nc.main_func.blocks:
        old = list(bb.instructions)
        need = any(
            ins.sync_info is not None and len(ins.sync_info.on_wait) > max_waits
            for ins in old
        )
        if not need:
            continue
        new = []
        for ins in old:
            si = ins.sync_info
            waits = list(si.on_wait) if si is not None else []
            if len(waits) > max_waits:
                chunks = [
                    waits[i : i + max_waits] for i in range(0, len(waits), max_waits)
                ]
                for ch in chunks[:-1]:
                    nop = mybir.InstNoOp(name=f"wsplit_{cnt}", ins=[], outs=[])
                    cnt += 1
                    nop.engine = ins.engine
                    nop.sync_info = bass_rust.SyncInfo(on_wait=ch, on_update=[])
                    new.append(nop)
                ins.sync_info = bass_rust.SyncInfo(
                    on_wait=chunks[-1], on_update=si.on_update
                )
            new.append(ins)
        bb.instructions = new
    return cnt


def _strip_const_memsets(nc):
    """The Bass preamble materializes 4 const APs via Pool memsets before the
    start barrier; this kernel reads none of them. Drop them so Pool reaches
    the barrier ~400ns sooner."""
    n = 0
    for bb in nc.main_func.blocks:
        keep = []
        for ins in bb.instructions:
            if (
                type(ins).__name__ == "InstMemset"
                and ins.outs
                and str(getattr(ins.outs[0], "memref", "")).startswith("const-")
                and ins.sync_info is None
            ):
                n += 1
                continue
            keep.append(ins)
        bb.instructions = keep
    return n


class _SplitDrainTileContext(tile.TileContext):
    def _drain_and_barrier(self, tick_clock, wait_clock):
        super()._drain_and_barrier(tick_clock, wait_clock)
        n = _split_excess_waits(self.nc)
        m = _strip_const_memsets(self.nc)
        print(f"[kernel] split {n} excess-wait chunks onto nops; "
              f"stripped {m} const memsets")


def _build_nc():
    nc = bass.Bass()
    blob_d = nc.declare_dram_parameter("blob", [128, NB], BF16, isOutput=False)
    out_d = nc.declare_dram_parameter("out", [SPC, 1], F32, isOutput=True)

    with _SplitDrainTileContext(nc) as tc:
        with (
            tc.tile_pool(name="sb", bufs=1) as sb,
            tc.tile_pool(name="ps", bufs=1, space="PSUM") as ps,
        ):
            blob = sb.tile([128, NB], BF16, tag="blob")
            nc.sync.dma_start(out=blob[:, 0:D0A_END], in_=blob_d[:, 0:D0A_END])
            nc.sync.dma_start(
                out=blob[:, D0A_END:D0B_END], in_=blob_d[:, D0A_END:D0B_END]
            )
            nc.sync.dma_start(out=blob[:, D0B_END:NB], in_=blob_d[:, D0B_END:NB])

            ones = blob[:, C_ONES : C_ONES + 1]
            onesr = sb.tile([1, 128], FP8, tag="onesr")
            nc.vector.memset(onesr, 1.0)
            onesrf = blob[0:1, C_ONESRF : C_ONESRF + 256].bitcast(F32)
            ones4b = blob[0:1, C_ONE4B : C_ONE4B + 4]

            pT = blob[:, C_PT : C_PT + 13].bitcast(FP8)

            psGA = ps.tile([58, 512], F32, tag="psGA")
            psGB = ps.tile([58, 512], F32, tag="psGB")
            psG = [psGA, psGB]
            psGt = ps.tile([128, SPC * NLT, PV], F32, tag="psGt")
            psM = ps.tile([128, 8], F32, tag="psM")    # pv 0:4 | dv 4:8
            psS = ps.tile([1, 2, 4], F32, tag="psS")   # pden [0,:] | dden [1,:]
            psZ = ps.tile([4, 65], F32, tag="psZ")
            psR = ps.tile([128, 8], F32, tag="psR")    # rec broadcast

            def g_mms(s):
                dfT = blob[:, C_DFT(s) : C_DFT(s) + 256].bitcast(FP8)
                off = 32 * (s % 2)
                nc.tensor.matmul(
                    psG[s // 2][off : off + PV, :], lhsT=pT, rhs=dfT,
                    start=True, stop=True,
                )

            def gt_mms(s):
                dfT = blob[:, C_DFT(s) : C_DFT(s) + 256].bitcast(FP8)
                for t in range(NLT):
                    nc.tensor.matmul(
                        psGt[:, NLT * s + t, :],
                        lhsT=dfT[:, 128 * t : 128 * (t + 1)],
                        rhs=pT,
                        start=True, stop=False,
                        skip_group_check=True,
                    )

            def gt_mask(s):
                nc.tensor.matmul(
                    psGt[:, NLT * s : NLT * (s + 1), :],
                    lhsT=onesr[:],
                    rhs=blob[0:1, C_MASK + 52 * s : C_MASK + 52 * (s + 1)].bitcast(
                        FP8
                    ),
                    start=False, stop=True,
                    skip_group_check=True,
                )

            # ---- PE: affinity matmuls. gt_mms(0) first: its 5 cheap matmuls
            # clog the 4-deep PE wait queue so every later matmul is costed
            # with a fully-ramped p-state clock.
            gt_mms(0)
            g_mms(0)
            g_mms(1)
            gt_mms(1)
            g_mms(2)
            g_mms(3)
            gt_mms(2)
            gt_mms(3)
            gt_mask(0)
            gt_mask(1)
            gt_mask(2)
            gt_mask(3)

            # ---- pair A head: LSE on ACT ----
            EA = sb.tile([58, 512], BF16, tag="EA")
            SA = sb.tile([58, 1], F32, tag="SA")
            zbias = blob[0:58, C_ZERO : C_ZERO + 2].bitcast(F32)
            nc.scalar.activation(
                EA, psGA[:, :], AF.Exp, scale=KLSE / SC, accum_out=SA,
                bias=zbias,
            )
            lnSA = sb.tile([58, 1], F32, tag="lnSA")
            nc.scalar.activation(lnSA, SA, AF.Ln, bias=zbias)
            epA = sb.tile([58, 1], BF16, tag="epA")
            nc.scalar.activation(
                epA, lnSA, AF.Exp, scale=1.0 / KLSE,
                bias=blob[0:58, C_LNC : C_LNC + 2].bitcast(F32),
            )
            # ---- DVE reduces (rmA early; mB right after G3; rmB last) ----
            rm = sb.tile([128, 4 * NLT], F32, tag="rm")
            nc.vector.reduce_max(
                rm[:, 0 : 2 * NLT], psGt[:, 0 : 2 * NLT, :],
                axis=mybir.AxisListType.X,
            )
            mB = sb.tile([58, 1], F32, tag="mB")
            nc.vector.reduce_max(mB, psGB[:, :], axis=mybir.AxisListType.X)
            nc.vector.reduce_max(
                rm[:, 2 * NLT : 4 * NLT], psGt[:, 2 * NLT : 4 * NLT, :],
                axis=mybir.AxisListType.X,
            )
            # ---- ACT exps ----
            epB = sb.tile([58, 1], BF16, tag="epB")
            nc.scalar.activation(
                epB, mB, AF.Exp, scale=1.0 / SC,
                bias=blob[0:58, C_LNC + 2 : C_LNC + 4].bitcast(F32),
            )
            ed = sb.tile([128, 4 * NLT], BF16, tag="ed")
            nc.scalar.activation(
                ed, rm, AF.Exp, scale=1.0 / SC,
                bias=blob[:, C_ZERO : C_ZERO + 2].bitcast(F32),
            )

            # ---- pools ----
            def pools(p, ep):
                for j in range(2):
                    s = 2 * p + j
                    off = 32 * j
                    nc.tensor.matmul(
                        psM[:, s : s + 1],
                        lhsT=blob[off : off + PV, C_PEMB : C_PEMB + 128],
                        rhs=ep[off : off + PV, 0:1],
                        start=True, stop=True,
                    )
                    nc.tensor.matmul(
                        psS[0:1, 0, s : s + 1],
                        lhsT=ep[off : off + PV, 0:1],
                        rhs=ones[off : off + PV, 0:1],
                        start=True, stop=True,
                    )
                    for t in range(NLT):
                        nc.tensor.matmul(
                            psM[:, 4 + s : 5 + s],
                            lhsT=blob[:, C_DFN(s) + 128 * t : C_DFN(s) + 128 * (t + 1)],
                            rhs=ed[:, NLT * s + t : NLT * s + t + 1],
                            start=(t == 0), stop=(t == NLT - 1),
                        )
                ed3 = ed[:, :].rearrange("p (s t) -> p s t", t=NLT)
                for t in range(NLT):
                    nc.tensor.matmul(
                        psS[0:1, 1, 2 * p : 2 * p + 2],
                        lhsT=ones[:], rhs=ed3[:, 2 * p : 2 * p + 2, t],
                        start=(t == 0), stop=(t == NLT - 1),
                    )

            pools(0, epA)
            pools(1, epB)

            # ---- tail: dsum -> recips -> broadcast -> normalized cv ->
            #      zT matmul (W1*|w2| + b1*|w2|, col 64 = |b2|) ->
            #      fused relu+signed-dot via stt accumulate -> DMA ----
            rec8 = sb.tile([1, 8], F32, tag="rec8")
            nc.vector.reciprocal(rec8, psS[0:1, 0:2, :])
            nc.tensor.matmul(
                psR[:, 0:8], lhsT=onesrf, rhs=rec8[:], start=True, stop=True,
            )
            cv = sb.tile([128, 8], F32, tag="cv")
            nc.vector.tensor_scalar(
                out=cv, in0=psM[:, :], scalar1=1.0, scalar2=None, op0=ALU.mult
            )
            cvn = sb.tile([128, 8], BF16, tag="cvn")
            nc.vector.tensor_tensor(
                out=cvn, in0=cv, in1=psR[:, :], op=ALU.mult
            )
            nc.tensor.matmul(
                psZ[:, :], lhsT=cvn[:, 4:8], rhs=blob[:, C_W1 : C_W1 + 65],
                start=True, stop=False,
            )
            nc.tensor.matmul(
                psZ[:, :], lhsT=cvn[:, 0:4],
                rhs=blob[:, C_W1 + 65 : C_W1 + 130],
                start=False, stop=False,
            )
            nc.tensor.matmul(
                psZ[:, :], lhsT=ones4b,
                rhs=blob[0:1, C_B1R : C_B1R + 65],
                start=False, stop=True,
            )
            dum = sb.tile([4, 65], BF16, tag="dum")
            tout = sb.tile([4, 1], F32, tag="tout")
            nc.vector.scalar_tensor_tensor(
                out=dum, in0=psZ[:, :], scalar=0.0,
                in1=blob[0:4, C_YROW : C_YROW + 65],
                op0=ALU.max, op1=ALU.mult,
                accum_out=tout,
            )
            nc.sync.dma_start(out=out_d[:], in_=tout)
    return nc
    return nc


_NC_CACHE = None


def _pack_blob(drug_ids, prot_ids, drug_emb, prot_emb, W1, b1, W2, b2):
    bf = ml_dtypes.bfloat16
    f8 = ml_dtypes.float8_e4m3
    d_feat = drug_emb[drug_ids]                       # [B, LD, H] f32
    dfT = np.ascontiguousarray(d_feat.transpose(0, 2, 1))
    dfT8 = (dfT * DS).astype(f8)                      # [B, 128, 512] fp8
    dfn = np.ascontiguousarray(
        d_feat.reshape(B, NLT, 128, H).transpose(0, 2, 1, 3).reshape(B, 128, NLT * H)
    ).astype(bf)
    counts = np.zeros((B, PV), np.float32)
    for bi in range(B):
        counts[bi] = np.bincount(prot_ids[bi].astype(np.int64), minlength=PV)[:PV]
    lnc = np.where(counts > 0, np.log(np.maximum(counts, 1.0)), -30.0).astype(
        np.float32
    )
    maskb = np.where(counts > 0, 0.0, -300.0).astype(np.float32)

    def f32_as_bf16(a):
        return np.ascontiguousarray(a.astype(np.float32)).view(bf)

    def f8_as_bf16(a):
        return np.ascontiguousarray(a).view(np.uint8).view(np.uint16).view(bf)

    # fold |w2| into W1/b1; keep signs in yrow; col 64 carries b2
    w2 = W1[0:0]  # placeholder silence
    aw2 = np.abs(W2[:, 0])                            # [64]
    W1s = W1 * aw2[None, :]                           # [256, 64]
    b1s = b1 * aw2                                    # [64]
    yr = np.sign(W2[:, 0])                            # [64]

    blob = np.zeros((NCORES, 128, NB), dtype=bf)
    pT8 = (np.ascontiguousarray(prot_emb.T) * DS).astype(f8)   # [128, 26]
    pembn = prot_emb.astype(bf)
    for c in range(NCORES):
        bl = blob[c]
        s0 = SPC * c
        bl[:, C_PT : C_PT + 13] = f8_as_bf16(pT8)
        bl[0:PV, C_PEMB : C_PEMB + 128] = pembn
        bl[32 : 32 + PV, C_PEMB : C_PEMB + 128] = pembn
        bl[:, C_W1 : C_W1 + 64] = W1s[0:128].astype(bf)
        bl[:, C_W1 + 65 : C_W1 + 129] = W1s[128:256].astype(bf)
        bl[0, C_B1R : C_B1R + 64] = b1s.astype(bf)
        bl[0, C_B1R + 64] = np.abs(np.float32(b2[0])).astype(bf)
        bl[0:4, C_YROW : C_YROW + 64] = np.broadcast_to(yr, (4, 64)).astype(bf)
        bl[0:4, C_YROW + 64] = np.sign(np.float32(b2[0])).astype(bf)
        bl[:, C_ONES] = np.array(1.0, dtype=bf)
        bl[0, C_ONESRF : C_ONESRF + 256] = f32_as_bf16(
            np.ones((1, 128), np.float32)
        ).reshape(256)
        bl[0, C_ONE4B : C_ONE4B + 4] = np.array(1.0, dtype=bf)
        for p in range(2):
            bl[0:PV, C_LNC + 2 * p : C_LNC + 2 * p + 2] = f32_as_bf16(
                lnc[s0 + 2 * p].reshape(PV, 1)
            )
            bl[32 : 32 + PV, C_LNC + 2 * p : C_LNC + 2 * p + 2] = f32_as_bf16(
                lnc[s0 + 2 * p + 1].reshape(PV, 1)
            )
        for s in range(SPC):
            bl[0, C_MASK + 52 * s : C_MASK + 52 * (s + 1)] = f8_as_bf16(
                np.tile(maskb[s0 + s], NLT).astype(f8).reshape(1, 104)
            ).reshape(52)
            bl[:, C_DFT(s) : C_DFT(s) + 256] = f8_as_bf16(dfT8[s0 + s])
            bl[:, C_DFN(s) : C_DFN(s) + 512] = dfn[s0 + s]
    return blob


def kernel(drug_ids, prot_ids, drug_emb, prot_emb, W1, b1, W2, b2):
    global _NC_CACHE
    drug_ids = np.asarray(drug_ids)
    prot_ids = np.asarray(prot_ids)
    drug_emb = np.asarray(drug_emb, dtype=np.float32)
    prot_emb = np.asarray(prot_emb, dtype=np.float32)
    W1 = np.asarray(W1, dtype=np.float32)
    b1 = np.asarray(b1, dtype=np.float32)
    W2 = np.asarray(W2, dtype=np.float32)
    b2 = np.asarray(b2, dtype=np.float32)

    blob = _pack_blob(drug_ids, prot_ids, drug_emb, prot_emb, W1, b1, W2, b2)

    if _NC_CACHE is None:
        _NC_CACHE = _build_nc()
    nc = _NC_CACHE

    in_maps = [{"blob": blob[c]} for c in range(NCORES)]
    trace = bool(os.environ.get("KERNEL_TRACE"))
    res = run_bass_kernel_spmd(nc, in_maps, list(range(NCORES)), trace=trace)
    kernel.last_result = res
    out = np.concatenate([res.results[c]["out"] for c in range(NCORES)], axis=0)
    return out.astype(np.float32)


kernel.last_result = None


# revision 3
# speedup vs baseline: 1.1055x; 1.0512x over previous
"""MCANet forward on 8 NeuronCores — vocab-factored exact algorithm, v3.

prot vocab is only 26, so aff[l,m] = G[pid_m, l] with G = prot_emb @ d_feat^T
([26, 512] per sample). Row/col maxes, softmaxes and pooled vectors follow
from G plus per-sample vocab counts c_v (host bincount):

  rowmax[l] = max_{v present} G[v, l]          (Gt orientation, DVE reduce)
  colmax[m] = M[pid_m],  M[v] = max_l G[v, l]
  p_vec = sum_v c_v e^{M_v} emb_v / sum_v c_v e^{M_v}
  d_vec = sum_l e^{rowmax_l} f_l / sum_l e^{rowmax_l}

v3:
 - dfT shipped as fp8e4 (x8 scaled; G is x64 scaled, exp scales folded)
 - sample pair A's M via ACT exp-accumulate LSE (k=1024), pair B via DVE
   reduce_max -> engines balanced
 - transposed MLP tail: relu scale-invariance relu(z/l) = relu(z)/l with
   l = Dd*Dp turns per-sample scalars into per-partition columns; the whole
   post-pool chain is 2 PE hops + DVE-only legs.
"""

import os
import sys

sys.path.insert(0, "/opt/trn_rl_repo")
_HERE = os.path.dirname(os.path.abspath(__file__))
if _HERE not in sys.path:
    sys.path.insert(0, _HERE)

import numpy as np
import ml_dtypes

import concourse.bass as bass
import concourse.tile as tile
from concourse import mybir
from concourse.bass_utils import run_bass_kernel_spmd

F32 = mybir.dt.float32
BF16 = mybir.dt.bfloat16
FP8 = mybir.dt.float8e4
AF = mybir.ActivationFunctionType
ALU = mybir.AluOpType

NCORES = 8
B, LD, LP, H, PV = 32, 512, 4096, 128, 26
SPC = B // NCORES   # 4 samples per core
NLT = LD // 128     # 4 l-tiles
DS = 8.0            # host scale on dfT and pT (fp8 denormal dodge)
SC = DS * DS        # G is SC * G_true
KLSE = 1024.0       # LSE sharpness in true-G units

# ---- blob column layout (bf16 columns) ----
# D0a section: what the G/Gt matmuls need
C_PT = 0            # [128, 13] = [128, 26] fp8 prot_emb^T * 8
C_LNC = 14          # [58, 4] ln(counts) fp32: pairA 14:16, pairB 16:18
C_ONES = 18         # [128, 1] bf16 ones column
C_ONE4B = 20        # [1@p0, 4] bf16 ones row
C_ZERO = 24         # [128, 2] zero f32 column (activation bias)
C_SMA_END = 32
# D0b section: pools + tail constants
C_PEMB = 1344       # [58, 128] prot_emb bf16 (partitions 0:26 and 32:58)
C_W1 = 1472         # [128, 130] W1 * |w2| (65 d-cols then 65 p-cols, col 64/129 pad)
C_B1R = 1602        # [1@p0, 65] bf16: b1 * |w2| with col 64 = |b2|
C_YROW = 1668       # [4, 65] bf16: sign(w2) row, col 64 = sign(b2)
C_ONESRF = 1734     # [1@p0, 256] = [1, 128] f32 ones row
C_MASK = 1990       # [1@p0, 208] fp8 mask rows (-300 if absent), 52 per sample
C_SMB_END = 2198


def C_DFT(s):
    return C_SMA_END + 256 * s              # fp8: 256 bf16-cols = 512 vals


def C_DFN(s):
    return C_SMB_END + 512 * s


D0A_END = C_SMA_END + 256 * SPC   # smallA + all dfT
D0B_END = C_SMB_END               # + smallB
NB = C_SMB_END + 512 * SPC
_MAX_WAITS = int(os.environ.get("KERNEL_MAX_WAITS", "1"))


def _split_excess_waits(nc, max_waits=_MAX_WAITS):
    """Walrus rejects instructions with more than ~2 sync waits. Hoist excess
    waits onto injected same-engine NOPs immediately before the instruction."""
    import bass_rust

    cnt = 0
    for bb in nc.main_func.blocks:
        old = list(bb.instructions)
        need = any(
            ins.sync_info is not None and len(ins.sync_info.on_wait) > max_waits
            for ins in old
        )
        if not need:
            continue
        new = []
        for ins in old:
            si = ins.sync_info
            waits = list(si.on_wait) if si is not None else []
            if len(waits) > max_waits:
                chunks = [
                    waits[i : i + max_waits] for i in range(0, len(waits), max_waits)
                ]
                for ch in chunks[:-1]:
                    nop = mybir.InstNoOp(name=f"wsplit_{cnt}", ins=[], outs=[])
                    cnt += 1
                    nop.engine = ins.engine
                    nop.sync_info = bass_rust.SyncInfo(on_wait=ch, on_update=[])
                    new.append(nop)
                ins.sync_info = bass_rust.SyncInfo(
                    on_wait=chunks[-1], on_update=si.on_update
                )
            new.append(ins)
        bb.instructions = new
    return cnt


def _strip_const_memsets(nc):
    """The Bass preamble materializes 4 const APs via Pool memsets before the
    start barrier; this kernel reads none of them. Drop them so Pool reaches
    the barrier ~400ns sooner."""
    n = 0
    for bb in nc.main_func.blocks:
        keep = []
        for ins in bb.instructions:
            if (
                type(ins).__name__ == "InstMemset"
                and ins.outs
                and str(getattr(ins.outs[0], "memref", "")).startswith("const-")
                and ins.sync_info is None
            ):
                n += 1
                continue
            keep.append(ins)
        bb.instructions = keep
    return n


class _SplitDrainTileContext(tile.TileContext):
    def _drain_and_barrier(self, tick_clock, wait_clock):
        super()._drain_and_barrier(tick_clock, wait_clock)
        n = _split_excess_waits(self.nc)
        m = _strip_const_memsets(self.nc)
        print(f"[kernel] split {n} excess-wait chunks onto nops; "
              f"stripped {m} const memsets")


def _build_nc(need_mask=False):
    nc = bass.Bass()
    blob_d = nc.declare_dram_parameter("blob", [128, NB], BF16, isOutput=False)
    out_d = nc.declare_dram_parameter("out", [SPC, 1], F32, isOutput=True)

    with _SplitDrainTileContext(nc) as tc:
        with (
            tc.tile_pool(name="sb", bufs=1) as sb,
            tc.tile_pool(name="ps", bufs=1, space="PSUM") as ps,
        ):
            blob = sb.tile([128, NB], BF16, tag="blob")
            nc.sync.dma_start(out=blob[:, 0:D0A_END], in_=blob_d[:, 0:D0A_END])
            nc.sync.dma_start(
                out=blob[:, D0A_END:D0B_END], in_=blob_d[:, D0A_END:D0B_END]
            )
            nc.sync.dma_start(out=blob[:, D0B_END:NB], in_=blob_d[:, D0B_END:NB])

            ones = blob[:, C_ONES : C_ONES + 1]
            onesr = sb.tile([1, 128], FP8, tag="onesr")
            nc.vector.memset(onesr, 1.0)
            onesrf = blob[0:1, C_ONESRF : C_ONESRF + 256].bitcast(F32)
            ones4b = blob[0:1, C_ONE4B : C_ONE4B + 4]

            pT = blob[:, C_PT : C_PT + 13].bitcast(FP8)

            psGA = ps.tile([58, 512], F32, tag="psGA")
            psGB = ps.tile([58, 512], F32, tag="psGB")
            psG = [psGA, psGB]
            psGtA = ps.tile([128, 2 * NLT, PV], F32, tag="psGtA")
            psGtB = ps.tile([128, 2 * NLT, PV], F32, tag="psGtB")
            psGtP = [psGtA, psGtB]
            psM = ps.tile([128, 8], F32, tag="psM")    # pv 0:4 | dv 4:8
            psS = ps.tile([1, 2, 4], F32, tag="psS")   # pden [0,:] | dden [1,:]
            psZ = ps.tile([4, 65], F32, tag="psZ")
            psR = ps.tile([128, 8], F32, tag="psR")    # rec broadcast

            def g_mms(s):
                dfT = blob[:, C_DFT(s) : C_DFT(s) + 256].bitcast(FP8)
                off = 32 * (s % 2)
                nc.tensor.matmul(
                    psG[s // 2][off : off + PV, :], lhsT=pT, rhs=dfT,
                    start=True, stop=True,
                )

            def gt_mms(s):
                dfT = blob[:, C_DFT(s) : C_DFT(s) + 256].bitcast(FP8)
                psGt = psGtP[s // 2]
                for t in range(NLT):
                    nc.tensor.matmul(
                        psGt[:, NLT * (s % 2) + t, :],
                        lhsT=dfT[:, 128 * t : 128 * (t + 1)],
                        rhs=pT,
                        start=True,
                        stop=(not need_mask) and t == NLT - 1,
                        skip_group_check=True,
                    )

            def gt_mask(s):
                nc.tensor.matmul(
                    psGtP[s // 2][:, NLT * (s % 2) : NLT * (s % 2 + 1), :],
                    lhsT=onesr[:],
                    rhs=blob[0:1, C_MASK + 52 * s : C_MASK + 52 * (s + 1)].bitcast(
                        FP8
                    ),
                    start=False, stop=True,
                    skip_group_check=True,
                )

            # ---- PE: affinity matmuls. gt_mms(0) first: its 5 cheap matmuls
            # clog the 4-deep PE wait queue so every later matmul is costed
            # with a fully-ramped p-state clock.
            gt_mms(0)
            g_mms(0)
            g_mms(1)
            gt_mms(1)
            g_mms(2)
            g_mms(3)
            gt_mms(2)
            gt_mms(3)
            if need_mask:
                gt_mask(0)
                gt_mask(1)
                gt_mask(2)
                gt_mask(3)

            # ---- pair A head: LSE on ACT ----
            EA = sb.tile([58, 512], BF16, tag="EA")
            SA = sb.tile([58, 1], F32, tag="SA")
            zbias = blob[0:58, C_ZERO : C_ZERO + 2].bitcast(F32)
            nc.scalar.activation(
                EA, psGA[:, :], AF.Exp, scale=KLSE / SC, accum_out=SA,
                bias=zbias,
            )
            lnSA = sb.tile([58, 1], F32, tag="lnSA")
            nc.scalar.activation(lnSA, SA, AF.Ln, bias=zbias)
            epA = sb.tile([58, 1], BF16, tag="epA")
            nc.scalar.activation(
                epA, lnSA, AF.Exp, scale=1.0 / KLSE,
                bias=blob[0:58, C_LNC : C_LNC + 2].bitcast(F32),
            )
            # ---- DVE reduces (rmA early; mB right after G3; rmB last) ----
            rm = sb.tile([128, 4 * NLT], F32, tag="rm")
            nc.vector.reduce_max(
                rm[:, 0 : 2 * NLT], psGtA[:, :, :], axis=mybir.AxisListType.X
            )
            mB = sb.tile([58, 1], F32, tag="mB")
            nc.vector.reduce_max(mB, psGB[:, :], axis=mybir.AxisListType.X)
            nc.vector.reduce_max(
                rm[:, 2 * NLT : 4 * NLT], psGtB[:, :, :], axis=mybir.AxisListType.X
            )
            # ---- ACT exps ----
            epB = sb.tile([58, 1], BF16, tag="epB")
            nc.scalar.activation(
                epB, mB, AF.Exp, scale=1.0 / SC,
                bias=blob[0:58, C_LNC + 2 : C_LNC + 4].bitcast(F32),
            )
            ed = sb.tile([128, 4 * NLT], BF16, tag="ed")
            nc.scalar.activation(
                ed, rm, AF.Exp, scale=1.0 / SC,
                bias=blob[:, C_ZERO : C_ZERO + 2].bitcast(F32),
            )

            # ---- pools ----
            def pools(p, ep):
                for j in range(2):
                    s = 2 * p + j
                    off = 32 * j
                    nc.tensor.matmul(
                        psM[:, s : s + 1],
                        lhsT=blob[off : off + PV, C_PEMB : C_PEMB + 128],
                        rhs=ep[off : off + PV, 0:1],
                        start=True, stop=True,
                    )
                    nc.tensor.matmul(
                        psS[0:1, 0, s : s + 1],
                        lhsT=ep[off : off + PV, 0:1],
                        rhs=ones[off : off + PV, 0:1],
                        start=True, stop=True,
                    )
                ed3 = ed[:, :].rearrange("p (s t) -> p s t", t=NLT)
                for t in range(NLT):
                    nc.tensor.matmul(
                        psS[0:1, 1, 2 * p : 2 * p + 2],
                        lhsT=ones[:], rhs=ed3[:, 2 * p : 2 * p + 2, t],
                        start=(t == 0), stop=(t == NLT - 1),
                    )
                for j in range(2):
                    s = 2 * p + j
                    for t in range(NLT):
                        nc.tensor.matmul(
                            psM[:, 4 + s : 5 + s],
                            lhsT=blob[:, C_DFN(s) + 128 * t : C_DFN(s) + 128 * (t + 1)],
                            rhs=ed[:, NLT * s + t : NLT * s + t + 1],
                            start=(t == 0), stop=(t == NLT - 1),
                        )

            pools(0, epA)
            pools(1, epB)

            # ---- tail: dsum -> recips -> broadcast -> normalized cv ->
            #      zT matmul (W1*|w2| + b1*|w2|, col 64 = |b2|) ->
            #      fused relu+signed-dot via stt accumulate -> DMA ----
            rec8 = sb.tile([1, 8], F32, tag="rec8")
            nc.vector.reciprocal(rec8, psS[0:1, 0:2, :])
            nc.tensor.matmul(
                psR[:, 0:8], lhsT=onesrf, rhs=rec8[:], start=True, stop=True,
            )
            cv = sb.tile([128, 8], F32, tag="cv")
            nc.vector.tensor_scalar(
                out=cv, in0=psM[:, :], scalar1=1.0, scalar2=None, op0=ALU.mult
            )
            cvn = sb.tile([128, 8], BF16, tag="cvn")
            nc.vector.tensor_tensor(
                out=cvn, in0=cv, in1=psR[:, :], op=ALU.mult
            )
            nc.tensor.matmul(
                psZ[:, :], lhsT=cvn[:, 4:8], rhs=blob[:, C_W1 : C_W1 + 65],
                start=True, stop=False,
            )
            nc.tensor.matmul(
                psZ[:, :], lhsT=cvn[:, 0:4],
                rhs=blob[:, C_W1 + 65 : C_W1 + 130],
                start=False, stop=False,
            )
            nc.tensor.matmul(
                psZ[:, :], lhsT=ones4b,
                rhs=blob[0:1, C_B1R : C_B1R + 65],
                start=False, stop=True,
            )
            dum = sb.tile([4, 65], BF16, tag="dum")
            tout = sb.tile([4, 1], F32, tag="tout")
            nc.vector.scalar_tensor_tensor(
                out=dum, in0=psZ[:, :], scalar=0.0,
                in1=blob[0:4, C_YROW : C_YROW + 65],
                op0=ALU.max, op1=ALU.mult,
                accum_out=tout,
            )
            nc.sync.dma_start(out=out_d[:], in_=tout)
    return nc
    return nc


_NC_CACHE = None
_NC_MASKED = None


def _pack_blob(drug_ids, prot_ids, drug_emb, prot_emb, W1, b1, W2, b2):
    bf = ml_dtypes.bfloat16
    f8 = ml_dtypes.float8_e4m3
    d_feat = drug_emb[drug_ids]                       # [B, LD, H] f32
    dfT = np.ascontiguousarray(d_feat.transpose(0, 2, 1))
    dfT8 = (dfT * DS).astype(f8)                      # [B, 128, 512] fp8
    dfn = np.ascontiguousarray(
        d_feat.reshape(B, NLT, 128, H).transpose(0, 2, 1, 3).reshape(B, 128, NLT * H)
    ).astype(bf)
    counts = np.zeros((B, PV), np.float32)
    for bi in range(B):
        counts[bi] = np.bincount(prot_ids[bi].astype(np.int64), minlength=PV)[:PV]
    lnc = np.where(counts > 0, np.log(np.maximum(counts, 1.0)), -30.0).astype(
        np.float32
    )
    maskb = np.where(counts > 0, 0.0, -300.0).astype(np.float32)

    def f32_as_bf16(a):
        return np.ascontiguousarray(a.astype(np.float32)).view(bf)

    def f8_as_bf16(a):
        return np.ascontiguousarray(a).view(np.uint8).view(np.uint16).view(bf)

    # fold |w2| into W1/b1; keep signs in yrow; col 64 carries b2
    w2 = W1[0:0]  # placeholder silence
    aw2 = np.abs(W2[:, 0])                            # [64]
    W1s = W1 * aw2[None, :]                           # [256, 64]
    b1s = b1 * aw2                                    # [64]
    yr = np.sign(W2[:, 0])                            # [64]

    blob = np.zeros((NCORES, 128, NB), dtype=bf)
    pT8 = (np.ascontiguousarray(prot_emb.T) * DS).astype(f8)   # [128, 26]
    pembn = prot_emb.astype(bf)
    for c in range(NCORES):
        bl = blob[c]
        s0 = SPC * c
        bl[:, C_PT : C_PT + 13] = f8_as_bf16(pT8)
        bl[0:PV, C_PEMB : C_PEMB + 128] = pembn
        bl[32 : 32 + PV, C_PEMB : C_PEMB + 128] = pembn
        bl[:, C_W1 : C_W1 + 64] = W1s[0:128].astype(bf)
        bl[:, C_W1 + 65 : C_W1 + 129] = W1s[128:256].astype(bf)
        bl[0, C_B1R : C_B1R + 64] = b1s.astype(bf)
        bl[0, C_B1R + 64] = np.abs(np.float32(b2[0])).astype(bf)
        bl[0:4, C_YROW : C_YROW + 64] = np.broadcast_to(yr, (4, 64)).astype(bf)
        bl[0:4, C_YROW + 64] = np.sign(np.float32(b2[0])).astype(bf)
        bl[:, C_ONES] = np.array(1.0, dtype=bf)
        bl[0, C_ONESRF : C_ONESRF + 256] = f32_as_bf16(
            np.ones((1, 128), np.float32)
        ).reshape(256)
        bl[0, C_ONE4B : C_ONE4B + 4] = np.array(1.0, dtype=bf)
        for p in range(2):
            bl[0:PV, C_LNC + 2 * p : C_LNC + 2 * p + 2] = f32_as_bf16(
                lnc[s0 + 2 * p].reshape(PV, 1)
            )
            bl[32 : 32 + PV, C_LNC + 2 * p : C_LNC + 2 * p + 2] = f32_as_bf16(
                lnc[s0 + 2 * p + 1].reshape(PV, 1)
            )
        for s in range(SPC):
            bl[0, C_MASK + 52 * s : C_MASK + 52 * (s + 1)] = f8_as_bf16(
                np.tile(maskb[s0 + s], NLT).astype(f8).reshape(1, 104)
            ).reshape(52)
            bl[:, C_DFT(s) : C_DFT(s) + 256] = f8_as_bf16(dfT8[s0 + s])
            bl[:, C_DFN(s) : C_DFN(s) + 512] = dfn[s0 + s]
    return blob


def kernel(drug_ids, prot_ids, drug_emb, prot_emb, W1, b1, W2, b2):
    global _NC_CACHE
    drug_ids = np.asarray(drug_ids)
    prot_ids = np.asarray(prot_ids)
    drug_emb = np.asarray(drug_emb, dtype=np.float32)
    prot_emb = np.asarray(prot_emb, dtype=np.float32)
    W1 = np.asarray(W1, dtype=np.float32)
    b1 = np.asarray(b1, dtype=np.float32)
    W2 = np.asarray(W2, dtype=np.float32)
    b2 = np.asarray(b2, dtype=np.float32)

    blob = _pack_blob(drug_ids, prot_ids, drug_emb, prot_emb, W1, b1, W2, b2)
    cts = np.stack([
        np.bincount(prot_ids[bi].astype(np.int64), minlength=PV)[:PV]
        for bi in range(B)
    ])
    kernel._need_mask = bool((cts == 0).any())

    need_mask = bool(getattr(kernel, "_need_mask", False))
    global _NC_MASKED
    if _NC_CACHE is None or _NC_MASKED != need_mask:
        _NC_CACHE = _build_nc(need_mask)
        _NC_MASKED = need_mask
    nc = _NC_CACHE

    in_maps = [{"blob": blob[c]} for c in range(NCORES)]
    trace = bool(os.environ.get("KERNEL_TRACE"))
    res = run_bass_kernel_spmd(nc, in_maps, list(range(NCORES)), trace=trace)
    kernel.last_result = res
    out = np.concatenate([res.results[c]["out"] for c in range(NCORES)], axis=0)
    return out.astype(np.float32)


kernel.last_result = None


# revision 4
# speedup vs baseline: 1.1250x; 1.0176x over previous
"""MCANet forward on 8 NeuronCores — vocab-factored exact algorithm, v3.

prot vocab is only 26, so aff[l,m] = G[pid_m, l] with G = prot_emb @ d_feat^T
([26, 512] per sample). Row/col maxes, softmaxes and pooled vectors follow
from G plus per-sample vocab counts c_v (host bincount):

  rowmax[l] = max_{v present} G[v, l]          (Gt orientation, DVE reduce)
  colmax[m] = M[pid_m],  M[v] = max_l G[v, l]
  p_vec = sum_v c_v e^{M_v} emb_v / sum_v c_v e^{M_v}
  d_vec = sum_l e^{rowmax_l} f_l / sum_l e^{rowmax_l}

v3:
 - dfT shipped as fp8e4 (x8 scaled; G is x64 scaled, exp scales folded)
 - sample pair A's M via ACT exp-accumulate LSE (k=1024), pair B via DVE
   reduce_max -> engines balanced
 - transposed MLP tail: relu scale-invariance relu(z/l) = relu(z)/l with
   l = Dd*Dp turns per-sample scalars into per-partition columns; the whole
   post-pool chain is 2 PE hops + DVE-only legs.
"""

import os
import sys

sys.path.insert(0, "/opt/trn_rl_repo")
_HERE = os.path.dirname(os.path.abspath(__file__))
if _HERE not in sys.path:
    sys.path.insert(0, _HERE)

import numpy as np
import ml_dtypes

import concourse.bass as bass
import concourse.tile as tile
from concourse import mybir
from concourse.bass_utils import run_bass_kernel_spmd

F32 = mybir.dt.float32
BF16 = mybir.dt.bfloat16
FP8 = mybir.dt.float8e4
AF = mybir.ActivationFunctionType
ALU = mybir.AluOpType

NCORES = 8
B, LD, LP, H, PV = 32, 512, 4096, 128, 26
SPC = B // NCORES   # 4 samples per core
NLT = LD // 128     # 4 l-tiles
DS = 8.0            # host scale on dfT and pT (fp8 denormal dodge)
SC = DS * DS        # G is SC * G_true
KLSE = 1024.0       # LSE sharpness in true-G units

# ---- blob column layout (bf16 columns) ----
# D0a section: what the G/Gt matmuls need
C_PT = 0            # [128, 13] = [128, 26] fp8 prot_emb^T * 8
C_LNC = 14          # [58, 4] ln(counts) fp32: pairA 14:16, pairB 16:18
C_ONES = 18         # [128, 1] bf16 ones column
C_ONE4B = 20        # [1@p0, 4] bf16 ones row
C_ZERO = 24         # [128, 2] zero f32 column (activation bias)
C_SMA_END = 32
# D0b section: pools + tail constants
C_PEMB = 1344       # [58, 128] prot_emb bf16 (partitions 0:26 and 32:58)
C_W1 = 1472         # [128, 130] W1 * |w2| (65 d-cols then 65 p-cols, col 64/129 pad)
C_B1R = 1602        # [1@p0, 65] bf16: b1 * |w2| with col 64 = |b2|
C_YROW = 1668       # [4, 65] bf16: sign(w2) row, col 64 = sign(b2)
C_ONESRF = 1734     # [1@p0, 256] = [1, 128] f32 ones row
C_MASK = 1990       # [1@p0, 208] fp8 mask rows (-300 if absent), 52 per sample
C_SMB_END = 2198


def C_DFT(s):
    return C_SMA_END + 256 * s              # fp8: 256 bf16-cols = 512 vals


def C_DFN(s):
    return C_SMB_END + 512 * s


D0A_END = C_SMA_END + 256 * SPC   # smallA + all dfT
D0B_END = C_SMB_END               # + smallB
NB = C_SMB_END + 512 * SPC
_MAX_WAITS = int(os.environ.get("KERNEL_MAX_WAITS", "1"))


def _split_excess_waits(nc, max_waits=_MAX_WAITS):
    """Walrus rejects instructions with more than ~2 sync waits. Hoist excess
    waits onto injected same-engine NOPs immediately before the instruction."""
    import bass_rust

    cnt = 0
    for bb in nc.main_func.blocks:
        old = list(bb.instructions)
        need = any(
            ins.sync_info is not None and len(ins.sync_info.on_wait) > max_waits
            for ins in old
        )
        if not need:
            continue
        new = []
        for ins in old:
            si = ins.sync_info
            waits = list(si.on_wait) if si is not None else []
            if len(waits) > max_waits:
                chunks = [
                    waits[i : i + max_waits] for i in range(0, len(waits), max_waits)
                ]
                for ch in chunks[:-1]:
                    nop = mybir.InstNoOp(name=f"wsplit_{cnt}", ins=[], outs=[])
                    cnt += 1
                    nop.engine = ins.engine
                    nop.sync_info = bass_rust.SyncInfo(on_wait=ch, on_update=[])
                    new.append(nop)
                ins.sync_info = bass_rust.SyncInfo(
                    on_wait=chunks[-1], on_update=si.on_update
                )
            new.append(ins)
        bb.instructions = new
    return cnt


def _strip_preamble_regmoves(nc):
    """Drop the per-engine zero/bcreg preamble RegisterMoves: this kernel has
    no control flow and nothing reads them; they delay the start barrier."""
    n = 0
    for bb in nc.main_func.blocks:
        keep = []
        for ins in bb.instructions:
            if type(ins).__name__ == "InstRegisterMove" and ins.sync_info is None:
                rr = str(getattr(ins.outs[0], "regref", "")) if ins.outs else ""
                if rr.endswith("_zero") or "_bcreg" in rr:
                    n += 1
                    continue
            keep.append(ins)
        bb.instructions = keep
    return n


def _strip_const_memsets(nc):
    """The Bass preamble materializes 4 const APs via Pool memsets before the
    start barrier; this kernel reads none of them. Drop them so Pool reaches
    the barrier ~400ns sooner."""
    n = 0
    for bb in nc.main_func.blocks:
        keep = []
        for ins in bb.instructions:
            if (
                type(ins).__name__ == "InstMemset"
                and ins.outs
                and str(getattr(ins.outs[0], "memref", "")).startswith("const-")
                and ins.sync_info is None
            ):
                n += 1
                continue
            keep.append(ins)
        bb.instructions = keep
    return n


class _SplitDrainTileContext(tile.TileContext):
    def _drain_and_barrier(self, tick_clock, wait_clock):
        super()._drain_and_barrier(tick_clock, wait_clock)
        n = _split_excess_waits(self.nc)
        m = _strip_const_memsets(self.nc) + _strip_preamble_regmoves(self.nc)
        print(f"[kernel] split {n} excess-wait chunks onto nops; "
              f"stripped {m} const memsets")


def _build_nc(need_mask=False):
    nc = bass.Bass()
    blob_d = nc.declare_dram_parameter("blob", [128, NB], BF16, isOutput=False)
    out_d = nc.declare_dram_parameter("out", [SPC, 1], F32, isOutput=True)

    with _SplitDrainTileContext(nc) as tc:
        with (
            tc.tile_pool(name="sb", bufs=1) as sb,
            tc.tile_pool(name="ps", bufs=1, space="PSUM") as ps,
        ):
            blob = sb.tile([128, NB], BF16, tag="blob")
            nc.sync.dma_start(out=blob[:, 0:D0A_END], in_=blob_d[:, 0:D0A_END])
            nc.sync.dma_start(
                out=blob[:, D0A_END:D0B_END], in_=blob_d[:, D0A_END:D0B_END]
            )
            nc.sync.dma_start(out=blob[:, D0B_END:NB], in_=blob_d[:, D0B_END:NB])

            ones = blob[:, C_ONES : C_ONES + 1]
            onesr = sb.tile([1, 128], FP8, tag="onesr")
            nc.vector.memset(onesr, 1.0)
            onesrf = blob[0:1, C_ONESRF : C_ONESRF + 256].bitcast(F32)
            ones4b = blob[0:1, C_ONE4B : C_ONE4B + 4]

            pT = blob[:, C_PT : C_PT + 13].bitcast(FP8)

            psGA = ps.tile([58, 512], F32, tag="psGA")
            psGB = ps.tile([58, 512], F32, tag="psGB")
            psG = [psGA, psGB]
            psGtA = ps.tile([128, 2 * NLT, PV], F32, tag="psGtA")
            psGtB = ps.tile([128, 2 * NLT, PV], F32, tag="psGtB")
            psGtP = [psGtA, psGtB]
            psM = ps.tile([128, 8], F32, tag="psM")    # pv 0:4 | dv 4:8
            psS = ps.tile([1, 2, 4], F32, tag="psS")   # pden [0,:] | dden [1,:]
            psZ = ps.tile([4, 65], F32, tag="psZ")
            psR = ps.tile([128, 8], F32, tag="psR")    # rec broadcast

            def g_mms(s):
                dfT = blob[:, C_DFT(s) : C_DFT(s) + 256].bitcast(FP8)
                off = 32 * (s % 2)
                nc.tensor.matmul(
                    psG[s // 2][off : off + PV, :], lhsT=pT, rhs=dfT,
                    start=True, stop=True,
                )

            def gt_mms(s):
                dfT = blob[:, C_DFT(s) : C_DFT(s) + 256].bitcast(FP8)
                psGt = psGtP[s // 2]
                for t in range(NLT):
                    nc.tensor.matmul(
                        psGt[:, NLT * (s % 2) + t, :],
                        lhsT=dfT[:, 128 * t : 128 * (t + 1)],
                        rhs=pT,
                        start=True,
                        stop=(not need_mask) and t == NLT - 1,
                        skip_group_check=True,
                    )

            def gt_mask(s):
                nc.tensor.matmul(
                    psGtP[s // 2][:, NLT * (s % 2) : NLT * (s % 2 + 1), :],
                    lhsT=onesr[:],
                    rhs=blob[0:1, C_MASK + 52 * s : C_MASK + 52 * (s + 1)].bitcast(
                        FP8
                    ),
                    start=False, stop=True,
                    skip_group_check=True,
                )

            # ---- PE: affinity matmuls. gt_mms(0) first: its 5 cheap matmuls
            # clog the 4-deep PE wait queue so every later matmul is costed
            # with a fully-ramped p-state clock.
            gt_mms(0)
            g_mms(0)
            g_mms(1)
            gt_mms(1)
            g_mms(2)
            g_mms(3)
            gt_mms(2)
            gt_mms(3)
            if need_mask:
                gt_mask(0)
                gt_mask(1)
                gt_mask(2)
                gt_mask(3)

            # ---- pair A head: LSE on ACT ----
            EA = sb.tile([58, 512], BF16, tag="EA")
            SA = sb.tile([58, 1], F32, tag="SA")
            zbias = blob[0:58, C_ZERO : C_ZERO + 2].bitcast(F32)
            nc.scalar.activation(
                EA, psGA[:, :], AF.Exp, scale=KLSE / SC, accum_out=SA,
                bias=zbias,
            )
            lnSA = sb.tile([58, 1], F32, tag="lnSA")
            nc.scalar.activation(lnSA, SA, AF.Ln, bias=zbias)
            epA = sb.tile([58, 1], BF16, tag="epA")
            nc.scalar.activation(
                epA, lnSA, AF.Exp, scale=1.0 / KLSE,
                bias=blob[0:58, C_LNC : C_LNC + 2].bitcast(F32),
            )
            # ---- DVE reduces (rmA early; mB right after G3; rmB last) ----
            rm = sb.tile([128, 4 * NLT], F32, tag="rm")
            nc.vector.reduce_max(
                rm[:, 0 : 2 * NLT], psGtA[:, :, :], axis=mybir.AxisListType.X
            )
            mB = sb.tile([58, 1], F32, tag="mB")
            nc.vector.reduce_max(mB, psGB[:, :], axis=mybir.AxisListType.X)
            nc.vector.reduce_max(
                rm[:, 2 * NLT : 4 * NLT], psGtB[:, :, :], axis=mybir.AxisListType.X
            )
            # ---- ACT exps ----
            epB = sb.tile([58, 1], BF16, tag="epB")
            nc.scalar.activation(
                epB, mB, AF.Exp, scale=1.0 / SC,
                bias=blob[0:58, C_LNC + 2 : C_LNC + 4].bitcast(F32),
            )
            ed = sb.tile([128, 4 * NLT], BF16, tag="ed")
            nc.scalar.activation(
                ed, rm, AF.Exp, scale=1.0 / SC,
                bias=blob[:, C_ZERO : C_ZERO + 2].bitcast(F32),
            )

            # ---- pools ----
            def pools(p, ep):
                for j in range(2):
                    s = 2 * p + j
                    off = 32 * j
                    nc.tensor.matmul(
                        psM[:, s : s + 1],
                        lhsT=blob[off : off + PV, C_PEMB : C_PEMB + 128],
                        rhs=ep[off : off + PV, 0:1],
                        start=True, stop=True,
                    )
                    nc.tensor.matmul(
                        psS[0:1, 0, s : s + 1],
                        lhsT=ep[off : off + PV, 0:1],
                        rhs=ones[off : off + PV, 0:1],
                        start=True, stop=True,
                    )
                ed3 = ed[:, :].rearrange("p (s t) -> p s t", t=NLT)
                for t in range(NLT):
                    nc.tensor.matmul(
                        psS[0:1, 1, 2 * p : 2 * p + 2],
                        lhsT=ones[:], rhs=ed3[:, 2 * p : 2 * p + 2, t],
                        start=(t == 0), stop=(t == NLT - 1),
                    )
                for j in range(2):
                    s = 2 * p + j
                    for t in range(NLT):
                        nc.tensor.matmul(
                            psM[:, 4 + s : 5 + s],
                            lhsT=blob[:, C_DFN(s) + 128 * t : C_DFN(s) + 128 * (t + 1)],
                            rhs=ed[:, NLT * s + t : NLT * s + t + 1],
                            start=(t == 0), stop=(t == NLT - 1),
                        )

            pools(0, epA)
            pools(1, epB)

            # ---- tail: dsum -> recips -> broadcast -> normalized cv ->
            #      zT matmul (W1*|w2| + b1*|w2|, col 64 = |b2|) ->
            #      fused relu+signed-dot via stt accumulate -> DMA ----
            rec8 = sb.tile([1, 8], F32, tag="rec8")
            nc.vector.reciprocal(rec8, psS[0:1, 0:2, :])
            nc.tensor.matmul(
                psR[:, 0:8], lhsT=onesrf, rhs=rec8[:], start=True, stop=True,
            )
            cv = sb.tile([128, 8], F32, tag="cv")
            nc.scalar.copy(out=cv, in_=psM[:, :])
            cvn = sb.tile([128, 8], BF16, tag="cvn")
            nc.vector.tensor_tensor(
                out=cvn, in0=cv, in1=psR[:, :], op=ALU.mult
            )
            nc.tensor.matmul(
                psZ[:, :], lhsT=cvn[:, 4:8], rhs=blob[:, C_W1 : C_W1 + 65],
                start=True, stop=False,
            )
            nc.tensor.matmul(
                psZ[:, :], lhsT=cvn[:, 0:4],
                rhs=blob[:, C_W1 + 65 : C_W1 + 130],
                start=False, stop=False,
            )
            nc.tensor.matmul(
                psZ[:, :], lhsT=ones4b,
                rhs=blob[0:1, C_B1R : C_B1R + 65],
                start=False, stop=True,
            )
            dum = sb.tile([4, 65], BF16, tag="dum")
            tout = sb.tile([4, 1], F32, tag="tout")
            nc.vector.scalar_tensor_tensor(
                out=dum, in0=psZ[:, :], scalar=0.0,
                in1=blob[0:4, C_YROW : C_YROW + 65],
                op0=ALU.max, op1=ALU.mult,
                accum_out=tout,
            )
            nc.sync.dma_start(out=out_d[:], in_=tout)
    return nc
    return nc


_NC_CACHE = None
_NC_MASKED = None


def _pack_blob(drug_ids, prot_ids, drug_emb, prot_emb, W1, b1, W2, b2):
    bf = ml_dtypes.bfloat16
    f8 = ml_dtypes.float8_e4m3
    d_feat = drug_emb[drug_ids]                       # [B, LD, H] f32
    dfT = np.ascontiguousarray(d_feat.transpose(0, 2, 1))
    dfT8 = (dfT * DS).astype(f8)                      # [B, 128, 512] fp8
    dfn = np.ascontiguousarray(
        d_feat.reshape(B, NLT, 128, H).transpose(0, 2, 1, 3).reshape(B, 128, NLT * H)
    ).astype(bf)
    counts = np.zeros((B, PV), np.float32)
    for bi in range(B):
        counts[bi] = np.bincount(prot_ids[bi].astype(np.int64), minlength=PV)[:PV]
    lnc = np.where(counts > 0, np.log(np.maximum(counts, 1.0)), -30.0).astype(
        np.float32
    )
    maskb = np.where(counts > 0, 0.0, -300.0).astype(np.float32)

    def f32_as_bf16(a):
        return np.ascontiguousarray(a.astype(np.float32)).view(bf)

    def f8_as_bf16(a):
        return np.ascontiguousarray(a).view(np.uint8).view(np.uint16).view(bf)

    # fold |w2| into W1/b1; keep signs in yrow; col 64 carries b2
    w2 = W1[0:0]  # placeholder silence
    aw2 = np.abs(W2[:, 0])                            # [64]
    W1s = W1 * aw2[None, :]                           # [256, 64]
    b1s = b1 * aw2                                    # [64]
    yr = np.sign(W2[:, 0])                            # [64]

    blob = np.zeros((NCORES, 128, NB), dtype=bf)
    pT8 = (np.ascontiguousarray(prot_emb.T) * DS).astype(f8)   # [128, 26]
    pembn = prot_emb.astype(bf)
    for c in range(NCORES):
        bl = blob[c]
        s0 = SPC * c
        bl[:, C_PT : C_PT + 13] = f8_as_bf16(pT8)
        bl[0:PV, C_PEMB : C_PEMB + 128] = pembn
        bl[32 : 32 + PV, C_PEMB : C_PEMB + 128] = pembn
        bl[:, C_W1 : C_W1 + 64] = W1s[0:128].astype(bf)
        bl[:, C_W1 + 65 : C_W1 + 129] = W1s[128:256].astype(bf)
        bl[0, C_B1R : C_B1R + 64] = b1s.astype(bf)
        bl[0, C_B1R + 64] = np.abs(np.float32(b2[0])).astype(bf)
        bl[0:4, C_YROW : C_YROW + 64] = np.broadcast_to(yr, (4, 64)).astype(bf)
        bl[0:4, C_YROW + 64] = np.sign(np.float32(b2[0])).astype(bf)
        bl[:, C_ONES] = np.array(1.0, dtype=bf)
        bl[0, C_ONESRF : C_ONESRF + 256] = f32_as_bf16(
            np.ones((1, 128), np.float32)
        ).reshape(256)
        bl[0, C_ONE4B : C_ONE4B + 4] = np.array(1.0, dtype=bf)
        for p in range(2):
            bl[0:PV, C_LNC + 2 * p : C_LNC + 2 * p + 2] = f32_as_bf16(
                lnc[s0 + 2 * p].reshape(PV, 1)
            )
            bl[32 : 32 + PV, C_LNC + 2 * p : C_LNC + 2 * p + 2] = f32_as_bf16(
                lnc[s0 + 2 * p + 1].reshape(PV, 1)
            )
        for s in range(SPC):
            bl[0, C_MASK + 52 * s : C_MASK + 52 * (s + 1)] = f8_as_bf16(
                np.tile(maskb[s0 + s], NLT).astype(f8).reshape(1, 104)
            ).reshape(52)
            bl[:, C_DFT(s) : C_DFT(s) + 256] = f8_as_bf16(dfT8[s0 + s])
            bl[:, C_DFN(s) : C_DFN(s) + 512] = dfn[s0 + s]
    return blob


def kernel(drug_ids, prot_ids, drug_emb, prot_emb, W1, b1, W2, b2):
    global _NC_CACHE
    drug_ids = np.asarray(drug_ids)
    prot_ids = np.asarray(prot_ids)
    drug_emb = np.asarray(drug_emb, dtype=np.float32)
    prot_emb = np.asarray(prot_emb, dtype=np.float32)
    W1 = np.asarray(W1, dtype=np.float32)
    b1 = np.asarray(b1, dtype=np.float32)
    W2 = np.asarray(W2, dtype=np.float32)
    b2 = np.asarray(b2, dtype=np.float32)

    blob = _pack_blob(drug_ids, prot_ids, drug_emb, prot_emb, W1, b1, W2, b2)
    cts = np.stack([
        np.bincount(prot_ids[bi].astype(np.int64), minlength=PV)[:PV]
        for bi in range(B)
    ])
    kernel._need_mask = bool((cts == 0).any())

    need_mask = bool(getattr(kernel, "_need_mask", False))
    global _NC_MASKED
    if _NC_CACHE is None or _NC_MASKED != need_mask:
        _NC_CACHE = _build_nc(need_mask)
        _NC_MASKED = need_mask
    nc = _NC_CACHE

    in_maps = [{"blob": blob[c]} for c in range(NCORES)]
    trace = bool(os.environ.get("KERNEL_TRACE"))
    res = run_bass_kernel_spmd(nc, in_maps, list(range(NCORES)), trace=trace)
    kernel.last_result = res
    out = np.concatenate([res.results[c]["out"] for c in range(NCORES)], axis=0)
    return out.astype(np.float32)


kernel.last_result = None
